# revision 1
# baseline (speedup 1.0000x reference)
"""Trainium2 Bass kernel for nn_EnoughViT_63282048139394.

Key mathematical reduction (verified exactly against the reference):
  - Attention in this architecture mixes ONLY the batch dimension, per
    sequence position ("scores = einsum('sbe,sce->sbc')").  No operation
    mixes sequence positions.
  - The classifier reads ONLY the last position (the class token), and
    that position's initial value (class_token + pos[:, -1]) is identical
    for every batch element, so it stays identical through every layer
    (mean-over-batch of identical rows is the row; the score matrix is a
    constant; LN/MLP act per-token).
  - Therefore the full [64, 1000] output is 64 identical copies of a
    single-token forward pass which does not depend on `x` at all:
        u = class_token + pos[-1]
        for l in 12:  h  = LN1(u); a = h@Wv; sval = h.(h@Wtheta)
                      u  = h + a*(1 + sval/sqrt(E))
                      h2 = LN2(u); u = u + gelu(h2@W1+b1)@W2 + b2
        out = log_softmax(gelu(LN_f(u)@Wc1+bc1)@Wc2 + bc2)  broadcast to 64

The kernel streams the ~305MB of weights from HBM through SBUF and runs
the GEMV chain on the tensor engine (token stationary as lhsT, weights as
the moving operand).  GEMVs are 4-way column-tiled (tile_position col
groups) so four rhs streams run concurrently through the PE array.
"""

import numpy as np
from contextlib import ExitStack

import concourse.bass as bass
import concourse.tile as tile
from concourse import bacc, mybir
from concourse.bass_utils import run_bass_kernel_spmd

E = 768
HID = 3072
CLS = 1000
L = 12
EPS = 1e-5
INV_SQRT_E = 1.0 / float(np.sqrt(768.0))
DT = mybir.dt.float32
BF = mybir.dt.bfloat16
AX = mybir.AxisListType
OP = mybir.AluOpType
ACT = mybir.ActivationFunctionType
Q = 192      # quarter of a 768-wide GEMV output (4 col groups)
QC = 250     # quarter of the 1000-wide classifier output


def build_program(gelu_mode='hw', repeat=1, wdt=DT):
    nc = bacc.Bacc()

    inp = {}

    def din(name, shape, dt=DT):
        t = nc.dram_tensor(name, list(shape), dt, kind="ExternalInput")
        inp[name] = t
        return t

    for l in range(L):
        for c in range(2):
            din(f"wv{c}_{l}", (128, 3 * E), wdt)  # [p, s*768+n] = Wv[128(3c+s)+p, n]
            din(f"wt{c}_{l}", (128, 3 * E), wdt)
        for c in range(6):
            din(f"w1{c}_{l}", (128, HID), wdt)       # s = c
        for c in range(6):
            din(f"w2{c}_{l}", (128, 4 * E), wdt)     # s in 4c..4c+3
        din(f"vec{l}", (1, 5 * E))         # ln1_s, ln1_b, ln2_s, ln2_b, b2
        din(f"b1cm{l}", (128, 24))         # b1 in cm layout [p,s]=b1[128s+p]
    for c in range(6):
        din(f"wc1{c}", (128, HID), wdt)
    for c in range(8):
        din(f"wc2{c}", (128, 3 * CLS), wdt)
    din("fvec", (1, 2 * E + CLS))          # lnf_s, lnf_b, bc2
    din("bc1cm", (128, 24))
    din("u0", (1, E))

    out_t = nc.dram_tensor("out", [1, CLS], DT, kind="ExternalOutput")

    with ExitStack() as ctx:
        tc = ctx.enter_context(tile.TileContext(nc))
        wsm = ctx.enter_context(tc.tile_pool(name="wsm", bufs=2))
        wbg = ctx.enter_context(tc.tile_pool(name="wbg", bufs=6))
        vp = ctx.enter_context(tc.tile_pool(name="vp", bufs=2))
        pers = ctx.enter_context(tc.tile_pool(name="pers", bufs=1))
        wk = ctx.enter_context(tc.tile_pool(name="wk", bufs=1))
        ps_at = ctx.enter_context(tc.tile_pool(name="ps_at", bufs=1, space="PSUM"))
        ps_m = ctx.enter_context(tc.tile_pool(name="ps_m", bufs=3, space="PSUM"))
        ps_t = ctx.enter_context(tc.tile_pool(name="ps_t", bufs=1, space="PSUM"))

        epst = pers.tile([1, 1], DT)
        nc.vector.memset(epst[:], EPS)
        onet = pers.tile([1, 1], DT)
        nc.vector.memset(onet[:], 1.0)

        def gelu_out(x, out):
            if gelu_mode == 'hw':
                nc.scalar.activation(out=out[:], in_=x[:], func=ACT.Gelu)
                return
            y = wk.tile(list(x.shape), DT, tag="geluy")
            nc.vector.tensor_mul(y[:], x[:], x[:])
            nc.vector.tensor_scalar(
                out=y[:], in0=y[:], scalar1=0.044715, scalar2=1.0,
                op0=OP.mult, op1=OP.add)
            nc.vector.tensor_mul(y[:], y[:], x[:])
            nc.scalar.activation(out=y[:], in_=y[:], func=ACT.Tanh,
                                 scale=float(np.sqrt(2.0 / np.pi)))
            nc.vector.tensor_scalar(
                out=y[:], in0=y[:], scalar1=1.0, scalar2=0.5,
                op0=OP.add, op1=OP.mult)
            nc.vector.tensor_mul(out[:], x[:], y[:])

        def layer_norm(x_ap, s_ap, b_ap, out_tile):
            """out = (x - mean(x)) * rsqrt(var(x)+EPS) * s + b   (flat [1,E'])"""
            n = x_ap.shape[-1]
            scr = wk.tile([1, n], DT, tag="lnscr")
            scr2 = wk.tile([1, n], DT, tag="lnscr2")
            mean = wk.tile([1, 1], DT, tag="mean")
            msq = wk.tile([1, 1], DT, tag="msq")
            # mean on DVE; sum(x^2) on ACT — the two passes run concurrently
            nc.vector.tensor_scalar(
                out=scr[:], in0=x_ap, scalar1=1.0 / n, scalar2=None,
                op0=OP.mult, op1=OP.add, accum_out=mean[:])
            nc.scalar.activation(
                out=scr2[:], in_=x_ap, func=ACT.Square, accum_out=msq[:])
            mu2 = wk.tile([1, 1], DT, tag="mu2")
            nc.vector.tensor_scalar(
                out=mu2[:], in0=mean[:], scalar1=mean[:], scalar2=None, op0=OP.mult)
            var = wk.tile([1, 1], DT, tag="var")
            # var = sum(x^2)/n - mean^2
            nc.vector.tensor_scalar(
                out=var[:], in0=msq[:], scalar1=1.0 / n, scalar2=None, op0=OP.mult)
            nc.vector.tensor_sub(var[:], var[:], mu2[:])
            sd = wk.tile([1, 1], DT, tag="sd")
            nc.scalar.activation(out=sd[:], in_=var[:], func=ACT.Sqrt, bias=epst[:])
            rstd = wk.tile([1, 1], DT, tag="rstd")
            nc.vector.reciprocal(rstd[:], sd[:])
            nc.vector.tensor_scalar(
                out=out_tile[:], in0=x_ap, scalar1=mean[:], scalar2=rstd[:],
                op0=OP.subtract, op1=OP.mult)
            nc.vector.tensor_mul(out_tile[:], out_tile[:], s_ap)
            nc.vector.tensor_add(out_tile[:], out_tile[:], b_ap)

        def to_cm(flat_tile, n_seg, tag, dt=None):
            """[1, 128*n_seg] flat -> [128, n_seg] cm (cm[p,s]=flat[128s+p])."""
            ps = ps_t.tile([128, n_seg], DT, tag="tps")
            for s in range(n_seg):
                # out[p, s] = flat[128*s + p]: plain matmul, K=1, rhs=[[1.0]]
                nc.tensor.matmul(
                    ps[:, s:s + 1], flat_tile[0:1, 128 * s:128 * (s + 1)],
                    onet[:], start=True, stop=True)
            cm = wk.tile([128, n_seg], dt or wdt, tag=tag)
            nc.vector.tensor_copy(out=cm[:], in_=ps[:])
            return cm

        def mm_ct(pt, row, lhs_col, rhs_ap, start, stop):
            """col-tiled GEMV matmul: output [1, nn] at psum partition 32*row."""
            nc.tensor.matmul(
                pt[32 * row:32 * row + 1, 0:rhs_ap.shape[-1]], lhs_col, rhs_ap,
                start=start, stop=stop, tile_position=(0, 32 * row),
                skip_group_check=True)

        for _rep in range(repeat):
            u = pers.tile([1, E], DT)
            nc.sync.dma_start(out=u[:], in_=inp["u0"][:, :])

            def load_attn_vec(l):
                # small LN/bias vectors first: layer 0's LN1 must not wait
                # behind 4.5MB of attention-weight DMAs at kernel start
                vec = vp.tile([1, 5 * E], DT, tag="vec", name=f"vec{l}_t")
                nc.sync.dma_start(out=vec[:], in_=inp[f"vec{l}"][:, :])
                b1cm = vp.tile([128, 24], DT, tag="b1cm", name=f"b1cm{l}_t")
                nc.sync.dma_start(out=b1cm[:], in_=inp[f"b1cm{l}"][:, :])
                wv_, wt_ = [], []
                for c in range(2):
                    wvt = wsm.tile([128, 3 * E], wdt, tag="wv", name=f"wv{c}_{l}_t")
                    nc.sync.dma_start(out=wvt[:], in_=inp[f"wv{c}_{l}"][:, :])
                    wv_.append(wvt)
                    wtt = wsm.tile([128, 3 * E], wdt, tag="wt", name=f"wt{c}_{l}_t")
                    nc.sync.dma_start(out=wtt[:], in_=inp[f"wt{c}_{l}"][:, :])
                    wt_.append(wtt)
                return wv_, wt_, vec, b1cm

            nxt = load_attn_vec(0)
            for l in range(L):
                wv_, wt_, vec, b1cm = nxt
                w1c_ = []
                for c in range(6):
                    wti = wbg.tile([128, HID], wdt, tag="wb")
                    nc.sync.dma_start(out=wti[:], in_=inp[f"w1{c}_{l}"][:, :])
                    w1c_.append(wti)
                w2c_ = []
                for c in range(6):
                    wti = wbg.tile([128, 4 * E], wdt, tag="wb")
                    nc.sync.dma_start(out=wti[:], in_=inp[f"w2{c}_{l}"][:, :])
                    w2c_.append(wti)

                # ---- LN1 -> h ----
                h = wk.tile([1, E], DT, tag="h")
                layer_norm(u[:], vec[0:1, 0:E], vec[0:1, E:2 * E], h)
                hcm = to_cm(h, 6, "hcm")

                # ---- a = h@Wv, t = h@Wtheta (4-way col-tiled) ----
                psA = ps_at.tile([128, 512], DT, tag="pa")
                psB = ps_at.tile([128, 512], DT, tag="pb")
                for s in range(6):
                    st, sp = (s == 0), (s == 5)
                    lhs = hcm[:, s:s + 1]
                    c, sl = s // 3, s % 3
                    for g in range(4):
                        mm_ct(psA, g, lhs, wv_[c][:, sl * E + g * Q: sl * E + (g + 1) * Q], st, sp)
                    for g in range(4):
                        mm_ct(psB, g, lhs, wt_[c][:, sl * E + g * Q: sl * E + (g + 1) * Q], st, sp)

                if l + 1 < L:
                    nxt = load_attn_vec(l + 1)

                tflat = wk.tile([1, E], DT, tag="tflat")
                for g in range(4):
                    nc.scalar.copy(
                        out=tflat[0:1, g * Q:(g + 1) * Q], in_=psB[32 * g:32 * g + 1, 0:Q])

                # c0 = 1 + (h . t) / sqrt(E)
                scr = wk.tile([1, E], DT, tag="lnscr")
                sv = wk.tile([1, 1], DT, tag="sv")
                c0 = wk.tile([1, 1], DT, tag="c0")
                nc.vector.tensor_mul(scr[:], h[:], tflat[:])
                nc.vector.tensor_scalar(
                    out=scr[:], in0=scr[:], scalar1=INV_SQRT_E, scalar2=None,
                    op0=OP.mult, op1=OP.add, accum_out=sv[:])
                nc.vector.tensor_scalar(
                    out=c0[:], in0=sv[:], scalar1=1.0, scalar2=None, op0=OP.add)

                # u = h + a * c0
                for g in range(4):
                    nc.vector.tensor_scalar(
                        out=u[0:1, g * Q:(g + 1) * Q], in0=psA[32 * g:32 * g + 1, 0:Q],
                        scalar1=c0[:], scalar2=None, op0=OP.mult)
                nc.vector.tensor_add(u[:], u[:], h[:])

                # ---- LN2 -> h2 ----
                h2 = wk.tile([1, E], DT, tag="h2")
                layer_norm(u[:], vec[0:1, 2 * E:3 * E], vec[0:1, 3 * E:4 * E], h2)
                h2cm = to_cm(h2, 6, "h2cm")

                # ---- m1 = h2@W1: 6 n-tiles of 512 on col groups 0-3 / 0-1 ----
                psC = ps_m.tile([128, 512], DT, tag="m")
                psD = ps_m.tile([128, 512], DT, tag="m")
                for s in range(6):
                    st, sp = (s == 0), (s == 5)
                    lhs = h2cm[:, s:s + 1]
                    wsrc = w1c_[s]
                    sl = 0
                    for nt in range(6):
                        pt, row = (psC, nt) if nt < 4 else (psD, nt - 4)
                        mm_ct(pt, row, lhs,
                              wsrc[:, sl * HID + nt * 512: sl * HID + nt * 512 + 512],
                              st, sp)
                gflat = wk.tile([1, HID], DT, tag="gflat")
                for nt in range(6):
                    pt, row = (psC, nt) if nt < 4 else (psD, nt - 4)
                    eng = nc.scalar if nt % 2 == 0 else nc.vector
                    if nt % 2 == 0:
                        nc.scalar.copy(
                            out=gflat[0:1, nt * 512:(nt + 1) * 512],
                            in_=pt[32 * row:32 * row + 1, :])
                    else:
                        nc.vector.tensor_copy(
                            out=gflat[0:1, nt * 512:(nt + 1) * 512],
                            in_=pt[32 * row:32 * row + 1, :])
                gcm32 = to_cm(gflat, 24, "gcm32", dt=DT)
                nc.vector.tensor_add(gcm32[:], gcm32[:], b1cm[:])
                gcm = wk.tile([128, 24], wdt, tag="gcm")
                gelu_out(gcm32, gcm)

                # ---- m2 = g@W2 (4x192 col groups) ; u = u + m2 + b2 ----
                psE = ps_m.tile([128, 512], DT, tag="m")
                for s in range(24):
                    st, sp = (s == 0), (s == 23)
                    lhs = gcm[:, s:s + 1]
                    wsrc = w2c_[s // 4]
                    sl = s % 4
                    for g in range(4):
                        mm_ct(psE, g, lhs, wsrc[:, sl * E + g * Q: sl * E + (g + 1) * Q],
                              st, sp)
                for g in range(4):
                    nc.vector.tensor_add(
                        u[0:1, g * Q:(g + 1) * Q], u[0:1, g * Q:(g + 1) * Q],
                        psE[32 * g:32 * g + 1, 0:Q])
                nc.vector.tensor_add(u[:], u[:], vec[0:1, 4 * E:5 * E])

            # ---- classifier ----
            fvec = vp.tile([1, 2 * E + CLS], DT, tag="vec")
            nc.sync.dma_start(out=fvec[:], in_=inp["fvec"][:, :])
            bc1cm = vp.tile([128, 24], DT, tag="b1cm")
            nc.sync.dma_start(out=bc1cm[:], in_=inp["bc1cm"][:, :])

            cf = wk.tile([1, E], DT, tag="h")
            layer_norm(u[:], fvec[0:1, 0:E], fvec[0:1, E:2 * E], cf)
            cfcm = to_cm(cf, 6, "hcm")

            wc1c_ = []
            for c in range(6):
                wti = wbg.tile([128, HID], wdt, tag="wb")
                nc.sync.dma_start(out=wti[:], in_=inp[f"wc1{c}"][:, :])
                wc1c_.append(wti)
            psC = ps_m.tile([128, 512], DT, tag="m")
            psD = ps_m.tile([128, 512], DT, tag="m")
            for s in range(6):
                st, sp = (s == 0), (s == 5)
                lhs = cfcm[:, s:s + 1]
                wsrc = wc1c_[s]
                sl = 0
                for nt in range(6):
                    pt, row = (psC, nt) if nt < 4 else (psD, nt - 4)
                    mm_ct(pt, row, lhs,
                          wsrc[:, sl * HID + nt * 512: sl * HID + nt * 512 + 512],
                          st, sp)
            g2flat = wk.tile([1, HID], DT, tag="gflat")
            for nt in range(6):
                pt, row = (psC, nt) if nt < 4 else (psD, nt - 4)
                nc.vector.tensor_copy(
                    out=g2flat[0:1, nt * 512:(nt + 1) * 512],
                    in_=pt[32 * row:32 * row + 1, :])
            g2cm32 = to_cm(g2flat, 24, "gcm32", dt=DT)
            nc.vector.tensor_add(g2cm32[:], g2cm32[:], bc1cm[:])
            g2cm = wk.tile([128, 24], wdt, tag="gcm")
            gelu_out(g2cm32, g2cm)

            wc2 = []
            for c in range(8):
                w = wbg.tile([128, 3 * CLS], wdt, tag="wb")
                nc.sync.dma_start(out=w[:], in_=inp[f"wc2{c}"][:, :])
                wc2.append(w)
            psF = ps_m.tile([128, 512], DT, tag="m")
            for s in range(24):
                st, sp = (s == 0), (s == 23)
                lhs = g2cm[:, s:s + 1]
                wsrc = wc2[s // 3]
                sl = s % 3
                for g in range(4):
                    mm_ct(psF, g, lhs, wsrc[:, sl * CLS + g * QC: sl * CLS + (g + 1) * QC],
                          st, sp)
            lg = wk.tile([1, CLS], DT, tag="lg")
            for g in range(4):
                nc.vector.tensor_copy(
                    out=lg[0:1, g * QC:(g + 1) * QC], in_=psF[32 * g:32 * g + 1, 0:QC])
            nc.vector.tensor_add(lg[:], lg[:], fvec[0:1, 2 * E:2 * E + CLS])

            # log_softmax
            mx = wk.tile([1, 1], DT, tag="mx")
            nc.vector.reduce_max(mx[:], lg[:], axis=AX.X)
            sh = wk.tile([1, CLS], DT, tag="sh")
            nc.vector.tensor_scalar(
                out=sh[:], in0=lg[:], scalar1=mx[:], scalar2=None, op0=OP.subtract)
            se = wk.tile([1, 1], DT, tag="se")
            nc.scalar.activation(out=lg[:], in_=sh[:], func=ACT.Exp, accum_out=se[:])
            lse = wk.tile([1, 1], DT, tag="lse")
            nc.scalar.activation(out=lse[:], in_=se[:], func=ACT.Ln)
            nc.vector.tensor_scalar(
                out=sh[:], in0=sh[:], scalar1=lse[:], scalar2=None, op0=OP.subtract)
            nc.sync.dma_start(out=out_t[:, :], in_=sh[:])

    nc.compile()
    return nc


def prep_inputs(inputs, wnp=np.float32):
    """Numpy-side re-layout of the reference inputs into the DRAM tensors."""
    f32 = lambda x: np.ascontiguousarray(np.asarray(x, dtype=np.float32))
    fw = lambda x: np.ascontiguousarray(np.asarray(x, dtype=np.float32).astype(wnp))
    m = {}
    Wv, Wt = inputs["Wv"], inputs["Wtheta"]
    W1, W2 = inputs["W1"], inputs["W2"]
    for l in range(L):
        # cm contraction layout: tile[p, s*N + n] = W[128s + p, n]
        wv = np.asarray(Wv[l]).reshape(6, 128, E).transpose(1, 0, 2)
        wt = np.asarray(Wt[l]).reshape(6, 128, E).transpose(1, 0, 2)
        for c in range(2):
            m[f"wv{c}_{l}"] = fw(wv[:, 3 * c:3 * c + 3].reshape(128, 3 * E))
            m[f"wt{c}_{l}"] = fw(wt[:, 3 * c:3 * c + 3].reshape(128, 3 * E))
        w1 = np.asarray(W1[l]).reshape(6, 128, HID).transpose(1, 0, 2)
        for c in range(6):
            m[f"w1{c}_{l}"] = fw(w1[:, c].reshape(128, HID))
        w2 = np.asarray(W2[l]).reshape(24, 128, E).transpose(1, 0, 2)
        for c in range(6):
            m[f"w2{c}_{l}"] = fw(w2[:, 4 * c:4 * c + 4].reshape(128, 4 * E))
        m[f"vec{l}"] = f32(np.concatenate([
            inputs["ln1_s"][l], inputs["ln1_b"][l],
            inputs["ln2_s"][l], inputs["ln2_b"][l],
            inputs["b2"][l]])).reshape(1, 5 * E)
        m[f"b1cm{l}"] = f32(np.asarray(inputs["b1"][l]).reshape(24, 128).T)
    wc1 = np.asarray(inputs["Wc1"]).reshape(6, 128, HID).transpose(1, 0, 2)
    for c in range(6):
        m[f"wc1{c}"] = fw(wc1[:, c].reshape(128, HID))
    wc2 = np.asarray(inputs["Wc2"]).reshape(24, 128, CLS).transpose(1, 0, 2)
    for c in range(8):
        m[f"wc2{c}"] = fw(wc2[:, 3 * c:3 * c + 3].reshape(128, 3 * CLS))
    m["fvec"] = f32(np.concatenate([
        inputs["lnf_s"], inputs["lnf_b"], inputs["bc2"]])).reshape(1, 2 * E + CLS)
    m["bc1cm"] = f32(np.asarray(inputs["bc1"]).reshape(24, 128).T)
    u0 = np.asarray(inputs["class_token"]).reshape(E) + np.asarray(inputs["pos"]).reshape(-1, E)[-1]
    m["u0"] = f32(u0).reshape(1, E)
    return m


_CACHED = {}


def kernel(**inputs) -> np.ndarray:
    b = int(np.asarray(inputs["x"]).shape[0])
    in_map = prep_inputs(inputs)
    if "nc" not in _CACHED:
        _CACHED["nc"] = build_program()
    nc = _CACHED["nc"]
    r = run_bass_kernel_spmd(nc, [in_map], core_ids=[0])
    out = np.asarray(r.results[0]["out"]).reshape(1, CLS)
    return np.ascontiguousarray(np.broadcast_to(out, (b, CLS)).astype(np.float32))


if __name__ == "__main__":
    import time
    d = np.load("/root/problem/inputs_cache.npz")
    inputs = {k: d[k] for k in d.files}
    t0 = time.time()
    out = kernel(**inputs)
    print("kernel wall time:", time.time() - t0)
    exp = np.load("/root/problem/expected.npy")
    err = np.abs(out - exp).max()
    rel = err / np.abs(exp).max()
    print("absmax err:", err, "rel:", rel)



# revision 2
# speedup vs baseline: 1.7710x; 1.7710x over previous
"""Trainium2 Bass kernel for nn_EnoughViT_63282048139394.

Key mathematical reduction (verified exactly against the reference):
  - Attention in this architecture mixes ONLY the batch dimension, per
    sequence position ("scores = einsum('sbe,sce->sbc')").  No operation
    mixes sequence positions.
  - The classifier reads ONLY the last position (the class token), and
    that position's initial value (class_token + pos[:, -1]) is identical
    for every batch element, so it stays identical through every layer
    (mean-over-batch of identical rows is the row; the score matrix is a
    constant; LN/MLP act per-token).
  - Therefore the full [64, 1000] output is 64 identical copies of a
    single-token forward pass which does not depend on `x` at all:
        u = class_token + pos[-1]
        for l in 12:  h  = LN1(u); a = h@Wv; sval = h.(h@Wtheta)
                      u  = h + a*(1 + sval/sqrt(E))
                      h2 = LN2(u); u = u + gelu(h2@W1+b1)@W2 + b2
        out = log_softmax(gelu(LN_f(u)@Wc1+bc1)@Wc2 + bc2)  broadcast to 64

The kernel streams the ~305MB of weights from HBM through SBUF and runs
the GEMV chain on the tensor engine (token stationary as lhsT, weights as
the moving operand).  GEMVs are 4-way column-tiled (tile_position col
groups) so four rhs streams run concurrently through the PE array.
"""

import numpy as np
from contextlib import ExitStack

import concourse.bass as bass
import concourse.tile as tile
from concourse import bacc, mybir
from concourse.bass_utils import run_bass_kernel_spmd

E = 768
HID = 3072
CLS = 1000
L = 12
EPS = 1e-5
INV_SQRT_E = 1.0 / float(np.sqrt(768.0))
DT = mybir.dt.float32
BF = mybir.dt.bfloat16
AX = mybir.AxisListType
OP = mybir.AluOpType
ACT = mybir.ActivationFunctionType
Q = 192      # quarter of a 768-wide GEMV output (4 col groups)
QC = 250     # quarter of the 1000-wide classifier output


def build_program(gelu_mode='hw', repeat=1, wdt=BF):
    nc = bacc.Bacc()

    inp = {}

    def din(name, shape, dt=DT):
        t = nc.dram_tensor(name, list(shape), dt, kind="ExternalInput")
        inp[name] = t
        return t

    for l in range(L):
        for c in range(2):
            din(f"wv{c}_{l}", (128, 3 * E), wdt)  # [p, s*768+n] = Wv[128(3c+s)+p, n]
            din(f"wt{c}_{l}", (128, 3 * E), wdt)
        for c in range(6):
            din(f"w1{c}_{l}", (128, HID), wdt)       # s = c
        for c in range(6):
            din(f"w2{c}_{l}", (128, 4 * E), wdt)     # s in 4c..4c+3
        din(f"vec{l}", (1, 5 * E))         # ln1_s, ln1_b, ln2_s, ln2_b, b2
        din(f"b1cm{l}", (128, 24))         # b1 in cm layout [p,s]=b1[128s+p]
    for c in range(6):
        din(f"wc1{c}", (128, HID), wdt)
    for c in range(8):
        din(f"wc2{c}", (128, 3 * CLS), wdt)
    din("fvec", (1, 2 * E + CLS))          # lnf_s, lnf_b, bc2
    din("bc1cm", (128, 24))
    din("u0", (1, E))

    out_t = nc.dram_tensor("out", [1, CLS], DT, kind="ExternalOutput")

    with ExitStack() as ctx:
        tc = ctx.enter_context(tile.TileContext(nc))
        wsm = ctx.enter_context(tc.tile_pool(name="wsm", bufs=2))
        wbg = ctx.enter_context(tc.tile_pool(name="wbg", bufs=6))
        vp = ctx.enter_context(tc.tile_pool(name="vp", bufs=2))
        pers = ctx.enter_context(tc.tile_pool(name="pers", bufs=1))
        wk = ctx.enter_context(tc.tile_pool(name="wk", bufs=1))
        ps_at = ctx.enter_context(tc.tile_pool(name="ps_at", bufs=1, space="PSUM"))
        ps_m = ctx.enter_context(tc.tile_pool(name="ps_m", bufs=3, space="PSUM"))
        ps_t = ctx.enter_context(tc.tile_pool(name="ps_t", bufs=1, space="PSUM"))

        epst = pers.tile([1, 1], DT)
        nc.vector.memset(epst[:], EPS)
        onet = pers.tile([1, 1], DT)
        nc.vector.memset(onet[:], 1.0)

        def gelu_out(x, out):
            if gelu_mode == 'hw':
                nc.scalar.activation(out=out[:], in_=x[:], func=ACT.Gelu)
                return
            y = wk.tile(list(x.shape), DT, tag="geluy")
            nc.vector.tensor_mul(y[:], x[:], x[:])
            nc.vector.tensor_scalar(
                out=y[:], in0=y[:], scalar1=0.044715, scalar2=1.0,
                op0=OP.mult, op1=OP.add)
            nc.vector.tensor_mul(y[:], y[:], x[:])
            nc.scalar.activation(out=y[:], in_=y[:], func=ACT.Tanh,
                                 scale=float(np.sqrt(2.0 / np.pi)))
            nc.vector.tensor_scalar(
                out=y[:], in0=y[:], scalar1=1.0, scalar2=0.5,
                op0=OP.add, op1=OP.mult)
            nc.vector.tensor_mul(out[:], x[:], y[:])

        def layer_norm(x_ap, s_ap, b_ap, out_tile):
            """out = (x - mean(x)) * rsqrt(var(x)+EPS) * s + b   (flat [1,E'])"""
            n = x_ap.shape[-1]
            scr = wk.tile([1, n], DT, tag="lnscr")
            scr2 = wk.tile([1, n], DT, tag="lnscr2")
            mean = wk.tile([1, 1], DT, tag="mean")
            msq = wk.tile([1, 1], DT, tag="msq")
            # mean on DVE; sum(x^2) on ACT — the two passes run concurrently
            nc.vector.tensor_scalar(
                out=scr[:], in0=x_ap, scalar1=1.0 / n, scalar2=None,
                op0=OP.mult, op1=OP.add, accum_out=mean[:])
            nc.scalar.activation(
                out=scr2[:], in_=x_ap, func=ACT.Square, accum_out=msq[:])
            mu2 = wk.tile([1, 1], DT, tag="mu2")
            nc.vector.tensor_scalar(
                out=mu2[:], in0=mean[:], scalar1=mean[:], scalar2=None, op0=OP.mult)
            var = wk.tile([1, 1], DT, tag="var")
            # var = sum(x^2)/n - mean^2
            nc.vector.tensor_scalar(
                out=var[:], in0=msq[:], scalar1=1.0 / n, scalar2=None, op0=OP.mult)
            nc.vector.tensor_sub(var[:], var[:], mu2[:])
            sd = wk.tile([1, 1], DT, tag="sd")
            nc.scalar.activation(out=sd[:], in_=var[:], func=ACT.Sqrt, bias=epst[:])
            rstd = wk.tile([1, 1], DT, tag="rstd")
            nc.vector.reciprocal(rstd[:], sd[:])
            nc.vector.tensor_scalar(
                out=out_tile[:], in0=x_ap, scalar1=mean[:], scalar2=rstd[:],
                op0=OP.subtract, op1=OP.mult)
            nc.vector.tensor_mul(out_tile[:], out_tile[:], s_ap)
            nc.vector.tensor_add(out_tile[:], out_tile[:], b_ap)

        def to_cm(flat_tile, n_seg, tag, dt=None):
            """[1, 128*n_seg] flat -> [128, n_seg] cm (cm[p,s]=flat[128s+p])."""
            ps = ps_t.tile([128, n_seg], DT, tag="tps")
            for s in range(n_seg):
                # out[p, s] = flat[128*s + p]: plain matmul, K=1, rhs=[[1.0]]
                nc.tensor.matmul(
                    ps[:, s:s + 1], flat_tile[0:1, 128 * s:128 * (s + 1)],
                    onet[:], start=True, stop=True)
            cm = wk.tile([128, n_seg], dt or wdt, tag=tag)
            nc.vector.tensor_copy(out=cm[:], in_=ps[:])
            return cm

        def mm_ct(pt, row, lhs_col, rhs_ap, start, stop):
            """col-tiled GEMV matmul: output [1, nn] at psum partition 32*row."""
            nc.tensor.matmul(
                pt[32 * row:32 * row + 1, 0:rhs_ap.shape[-1]], lhs_col, rhs_ap,
                start=start, stop=stop, tile_position=(0, 32 * row),
                skip_group_check=True)

        for _rep in range(repeat):
            u = pers.tile([1, E], DT)
            nc.sync.dma_start(out=u[:], in_=inp["u0"][:, :])

            def load_attn_vec(l):
                # small LN/bias vectors first: layer 0's LN1 must not wait
                # behind 4.5MB of attention-weight DMAs at kernel start
                vec = vp.tile([1, 5 * E], DT, tag="vec", name=f"vec{l}_t")
                nc.sync.dma_start(out=vec[:], in_=inp[f"vec{l}"][:, :])
                b1cm = vp.tile([128, 24], DT, tag="b1cm", name=f"b1cm{l}_t")
                nc.sync.dma_start(out=b1cm[:], in_=inp[f"b1cm{l}"][:, :])
                wv_, wt_ = [], []
                for c in range(2):
                    wvt = wsm.tile([128, 3 * E], wdt, tag="wv", name=f"wv{c}_{l}_t")
                    nc.sync.dma_start(out=wvt[:], in_=inp[f"wv{c}_{l}"][:, :])
                    wv_.append(wvt)
                    wtt = wsm.tile([128, 3 * E], wdt, tag="wt", name=f"wt{c}_{l}_t")
                    nc.sync.dma_start(out=wtt[:], in_=inp[f"wt{c}_{l}"][:, :])
                    wt_.append(wtt)
                return wv_, wt_, vec, b1cm

            nxt = load_attn_vec(0)
            for l in range(L):
                wv_, wt_, vec, b1cm = nxt
                w1c_ = []
                for c in range(6):
                    wti = wbg.tile([128, HID], wdt, tag="wb")
                    nc.sync.dma_start(out=wti[:], in_=inp[f"w1{c}_{l}"][:, :])
                    w1c_.append(wti)
                w2c_ = []
                for c in range(6):
                    wti = wbg.tile([128, 4 * E], wdt, tag="wb")
                    nc.sync.dma_start(out=wti[:], in_=inp[f"w2{c}_{l}"][:, :])
                    w2c_.append(wti)

                # ---- LN1 -> h ----
                h = wk.tile([1, E], DT, tag="h")
                layer_norm(u[:], vec[0:1, 0:E], vec[0:1, E:2 * E], h)
                hcm = to_cm(h, 6, "hcm")

                # ---- a = h@Wv, t = h@Wtheta (4-way col-tiled) ----
                psA = ps_at.tile([128, 512], DT, tag="pa")
                psB = ps_at.tile([128, 512], DT, tag="pb")
                for s in range(6):
                    st, sp = (s == 0), (s == 5)
                    lhs = hcm[:, s:s + 1]
                    c, sl = s // 3, s % 3
                    for g in range(4):
                        mm_ct(psA, g, lhs, wv_[c][:, sl * E + g * Q: sl * E + (g + 1) * Q], st, sp)
                    for g in range(4):
                        mm_ct(psB, g, lhs, wt_[c][:, sl * E + g * Q: sl * E + (g + 1) * Q], st, sp)

                if l + 1 < L:
                    nxt = load_attn_vec(l + 1)

                tflat = wk.tile([1, E], DT, tag="tflat")
                for g in range(4):
                    nc.scalar.copy(
                        out=tflat[0:1, g * Q:(g + 1) * Q], in_=psB[32 * g:32 * g + 1, 0:Q])

                # c0 = 1 + (h . t) / sqrt(E)
                scr = wk.tile([1, E], DT, tag="lnscr")
                sv = wk.tile([1, 1], DT, tag="sv")
                c0 = wk.tile([1, 1], DT, tag="c0")
                nc.vector.tensor_mul(scr[:], h[:], tflat[:])
                nc.vector.tensor_scalar(
                    out=scr[:], in0=scr[:], scalar1=INV_SQRT_E, scalar2=None,
                    op0=OP.mult, op1=OP.add, accum_out=sv[:])
                nc.vector.tensor_scalar(
                    out=c0[:], in0=sv[:], scalar1=1.0, scalar2=None, op0=OP.add)

                # u = h + a * c0
                for g in range(4):
                    nc.vector.tensor_scalar(
                        out=u[0:1, g * Q:(g + 1) * Q], in0=psA[32 * g:32 * g + 1, 0:Q],
                        scalar1=c0[:], scalar2=None, op0=OP.mult)
                nc.vector.tensor_add(u[:], u[:], h[:])

                # ---- LN2 -> h2 ----
                h2 = wk.tile([1, E], DT, tag="h2")
                layer_norm(u[:], vec[0:1, 2 * E:3 * E], vec[0:1, 3 * E:4 * E], h2)
                h2cm = to_cm(h2, 6, "h2cm")

                # ---- m1 = h2@W1: 6 n-tiles of 512 on col groups 0-3 / 0-1 ----
                psC = ps_m.tile([128, 512], DT, tag="m")
                psD = ps_m.tile([128, 512], DT, tag="m")
                for s in range(6):
                    st, sp = (s == 0), (s == 5)
                    lhs = h2cm[:, s:s + 1]
                    wsrc = w1c_[s]
                    sl = 0
                    for nt in range(6):
                        pt, row = (psC, nt) if nt < 4 else (psD, nt - 4)
                        mm_ct(pt, row, lhs,
                              wsrc[:, sl * HID + nt * 512: sl * HID + nt * 512 + 512],
                              st, sp)
                gflat = wk.tile([1, HID], DT, tag="gflat")
                for nt in range(6):
                    pt, row = (psC, nt) if nt < 4 else (psD, nt - 4)
                    eng = nc.scalar if nt % 2 == 0 else nc.vector
                    if nt % 2 == 0:
                        nc.scalar.copy(
                            out=gflat[0:1, nt * 512:(nt + 1) * 512],
                            in_=pt[32 * row:32 * row + 1, :])
                    else:
                        nc.vector.tensor_copy(
                            out=gflat[0:1, nt * 512:(nt + 1) * 512],
                            in_=pt[32 * row:32 * row + 1, :])
                gcm32 = to_cm(gflat, 24, "gcm32", dt=DT)
                nc.vector.tensor_add(gcm32[:], gcm32[:], b1cm[:])
                gcm = wk.tile([128, 24], wdt, tag="gcm")
                gelu_out(gcm32, gcm)

                # ---- m2 = g@W2 (4x192 col groups) ; u = u + m2 + b2 ----
                psE = ps_m.tile([128, 512], DT, tag="m")
                for s in range(24):
                    st, sp = (s == 0), (s == 23)
                    lhs = gcm[:, s:s + 1]
                    wsrc = w2c_[s // 4]
                    sl = s % 4
                    for g in range(4):
                        mm_ct(psE, g, lhs, wsrc[:, sl * E + g * Q: sl * E + (g + 1) * Q],
                              st, sp)
                for g in range(4):
                    nc.vector.tensor_add(
                        u[0:1, g * Q:(g + 1) * Q], u[0:1, g * Q:(g + 1) * Q],
                        psE[32 * g:32 * g + 1, 0:Q])
                nc.vector.tensor_add(u[:], u[:], vec[0:1, 4 * E:5 * E])

            # ---- classifier ----
            fvec = vp.tile([1, 2 * E + CLS], DT, tag="vec")
            nc.sync.dma_start(out=fvec[:], in_=inp["fvec"][:, :])
            bc1cm = vp.tile([128, 24], DT, tag="b1cm")
            nc.sync.dma_start(out=bc1cm[:], in_=inp["bc1cm"][:, :])

            cf = wk.tile([1, E], DT, tag="h")
            layer_norm(u[:], fvec[0:1, 0:E], fvec[0:1, E:2 * E], cf)
            cfcm = to_cm(cf, 6, "hcm")

            wc1c_ = []
            for c in range(6):
                wti = wbg.tile([128, HID], wdt, tag="wb")
                nc.sync.dma_start(out=wti[:], in_=inp[f"wc1{c}"][:, :])
                wc1c_.append(wti)
            psC = ps_m.tile([128, 512], DT, tag="m")
            psD = ps_m.tile([128, 512], DT, tag="m")
            for s in range(6):
                st, sp = (s == 0), (s == 5)
                lhs = cfcm[:, s:s + 1]
                wsrc = wc1c_[s]
                sl = 0
                for nt in range(6):
                    pt, row = (psC, nt) if nt < 4 else (psD, nt - 4)
                    mm_ct(pt, row, lhs,
                          wsrc[:, sl * HID + nt * 512: sl * HID + nt * 512 + 512],
                          st, sp)
            g2flat = wk.tile([1, HID], DT, tag="gflat")
            for nt in range(6):
                pt, row = (psC, nt) if nt < 4 else (psD, nt - 4)
                nc.vector.tensor_copy(
                    out=g2flat[0:1, nt * 512:(nt + 1) * 512],
                    in_=pt[32 * row:32 * row + 1, :])
            g2cm32 = to_cm(g2flat, 24, "gcm32", dt=DT)
            nc.vector.tensor_add(g2cm32[:], g2cm32[:], bc1cm[:])
            g2cm = wk.tile([128, 24], wdt, tag="gcm")
            gelu_out(g2cm32, g2cm)

            wc2 = []
            for c in range(8):
                w = wbg.tile([128, 3 * CLS], wdt, tag="wb")
                nc.sync.dma_start(out=w[:], in_=inp[f"wc2{c}"][:, :])
                wc2.append(w)
            psF = ps_m.tile([128, 512], DT, tag="m")
            for s in range(24):
                st, sp = (s == 0), (s == 23)
                lhs = g2cm[:, s:s + 1]
                wsrc = wc2[s // 3]
                sl = s % 3
                for g in range(4):
                    mm_ct(psF, g, lhs, wsrc[:, sl * CLS + g * QC: sl * CLS + (g + 1) * QC],
                          st, sp)
            lg = wk.tile([1, CLS], DT, tag="lg")
            for g in range(4):
                nc.vector.tensor_copy(
                    out=lg[0:1, g * QC:(g + 1) * QC], in_=psF[32 * g:32 * g + 1, 0:QC])
            nc.vector.tensor_add(lg[:], lg[:], fvec[0:1, 2 * E:2 * E + CLS])

            # log_softmax
            mx = wk.tile([1, 1], DT, tag="mx")
            nc.vector.reduce_max(mx[:], lg[:], axis=AX.X)
            sh = wk.tile([1, CLS], DT, tag="sh")
            nc.vector.tensor_scalar(
                out=sh[:], in0=lg[:], scalar1=mx[:], scalar2=None, op0=OP.subtract)
            se = wk.tile([1, 1], DT, tag="se")
            nc.scalar.activation(out=lg[:], in_=sh[:], func=ACT.Exp, accum_out=se[:])
            lse = wk.tile([1, 1], DT, tag="lse")
            nc.scalar.activation(out=lse[:], in_=se[:], func=ACT.Ln)
            nc.vector.tensor_scalar(
                out=sh[:], in0=sh[:], scalar1=lse[:], scalar2=None, op0=OP.subtract)
            nc.sync.dma_start(out=out_t[:, :], in_=sh[:])

    nc.compile()
    return nc


import ml_dtypes


def prep_inputs(inputs, wnp=ml_dtypes.bfloat16):
    """Numpy-side re-layout of the reference inputs into the DRAM tensors."""
    f32 = lambda x: np.ascontiguousarray(np.asarray(x, dtype=np.float32))
    fw = lambda x: np.ascontiguousarray(np.asarray(x, dtype=np.float32).astype(wnp))
    m = {}
    Wv, Wt = inputs["Wv"], inputs["Wtheta"]
    W1, W2 = inputs["W1"], inputs["W2"]
    for l in range(L):
        # cm contraction layout: tile[p, s*N + n] = W[128s + p, n]
        wv = np.asarray(Wv[l]).reshape(6, 128, E).transpose(1, 0, 2)
        wt = np.asarray(Wt[l]).reshape(6, 128, E).transpose(1, 0, 2)
        for c in range(2):
            m[f"wv{c}_{l}"] = fw(wv[:, 3 * c:3 * c + 3].reshape(128, 3 * E))
            m[f"wt{c}_{l}"] = fw(wt[:, 3 * c:3 * c + 3].reshape(128, 3 * E))
        w1 = np.asarray(W1[l]).reshape(6, 128, HID).transpose(1, 0, 2)
        for c in range(6):
            m[f"w1{c}_{l}"] = fw(w1[:, c].reshape(128, HID))
        w2 = np.asarray(W2[l]).reshape(24, 128, E).transpose(1, 0, 2)
        for c in range(6):
            m[f"w2{c}_{l}"] = fw(w2[:, 4 * c:4 * c + 4].reshape(128, 4 * E))
        m[f"vec{l}"] = f32(np.concatenate([
            inputs["ln1_s"][l], inputs["ln1_b"][l],
            inputs["ln2_s"][l], inputs["ln2_b"][l],
            inputs["b2"][l]])).reshape(1, 5 * E)
        m[f"b1cm{l}"] = f32(np.asarray(inputs["b1"][l]).reshape(24, 128).T)
    wc1 = np.asarray(inputs["Wc1"]).reshape(6, 128, HID).transpose(1, 0, 2)
    for c in range(6):
        m[f"wc1{c}"] = fw(wc1[:, c].reshape(128, HID))
    wc2 = np.asarray(inputs["Wc2"]).reshape(24, 128, CLS).transpose(1, 0, 2)
    for c in range(8):
        m[f"wc2{c}"] = fw(wc2[:, 3 * c:3 * c + 3].reshape(128, 3 * CLS))
    m["fvec"] = f32(np.concatenate([
        inputs["lnf_s"], inputs["lnf_b"], inputs["bc2"]])).reshape(1, 2 * E + CLS)
    m["bc1cm"] = f32(np.asarray(inputs["bc1"]).reshape(24, 128).T)
    u0 = np.asarray(inputs["class_token"]).reshape(E) + np.asarray(inputs["pos"]).reshape(-1, E)[-1]
    m["u0"] = f32(u0).reshape(1, E)
    return m


_CACHED = {}


def kernel(**inputs) -> np.ndarray:
    b = int(np.asarray(inputs["x"]).shape[0])
    in_map = prep_inputs(inputs)
    if "nc" not in _CACHED:
        _CACHED["nc"] = build_program()
    nc = _CACHED["nc"]
    r = run_bass_kernel_spmd(nc, [in_map], core_ids=[0])
    out = np.asarray(r.results[0]["out"]).reshape(1, CLS)
    return np.ascontiguousarray(np.broadcast_to(out, (b, CLS)).astype(np.float32))


if __name__ == "__main__":
    import time
    d = np.load("/root/problem/inputs_cache.npz")
    inputs = {k: d[k] for k in d.files}
    t0 = time.time()
    out = kernel(**inputs)
    print("kernel wall time:", time.time() - t0)
    exp = np.load("/root/problem/expected.npy")
    err = np.abs(out - exp).max()
    rel = err / np.abs(exp).max()
    print("absmax err:", err, "rel:", rel)



# revision 3
# speedup vs baseline: 1.9470x; 1.0994x over previous
"""Trainium2 Bass kernel for nn_EnoughViT_63282048139394 — v2.

Same math reduction as v1 (single-token chain, batch-broadcast output), but:
  - activations live in column-major [128, segs] layout; LayerNorm stats via a
    ones-matmul partition reduce; affine applied with per-partition scalars
    (no 1-lane flat vector ops)
  - layout changes (flat<->cm) via PE transpose-mode, not K=1 matmul chains
  - W1/W2 stored as per-column-scaled float8_e3m4 (scales folded back on the
    CM side after the GEMV), attention + classifier weights bf16
  - activation-table thrash hidden by dummy Gelu/Sqrt ops issued while the
    tensor engine streams the next GEMV
"""

import numpy as np
import ml_dtypes
from contextlib import ExitStack

import concourse.bass as bass
import concourse.tile as tile
from concourse import bacc, mybir
from concourse.bass_utils import run_bass_kernel_spmd

E = 768
HID = 3072
CLS = 1000
L = 12
EPS = 1e-5
INV_SQRT_E = 1.0 / float(np.sqrt(768.0))
DT = mybir.dt.float32
BF = mybir.dt.bfloat16
F8 = mybir.dt.float8e3
AX = mybir.AxisListType
OP = mybir.AluOpType
ACT = mybir.ActivationFunctionType
F8LIM = 12.0


def build_program(gelu_mode='hw'):
    nc = bacc.Bacc()
    inp = {}

    def din(name, shape, dt=DT):
        t = nc.dram_tensor(name, list(shape), dt, kind="ExternalInput")
        inp[name] = t
        return t

    for l in range(L):
        for c in range(2):
            din(f"wv{c}_{l}", (128, 3 * E), BF)
            din(f"wt{c}_{l}", (128, 3 * E), BF)
        for c in range(6):
            din(f"w1{c}_{l}", (128, HID), F8)
        for c in range(6):
            din(f"w2{c}_{l}", (128, 4 * E), F8)
        din(f"pv{l}", (128, 84))   # ln1_s|ln1_b|ln2_s|ln2_b|s2|b2 (6 each), s1(24), b1(24)
    for c in range(6):
        din(f"wc1{c}", (128, HID), BF)
    for c in range(8):
        din(f"wc2{c}", (128, 3 * CLS), BF)
    din("fcm", (128, 36))          # lnf_s, lnf_b cm + bc1 cm
    din("fb", (1, CLS))            # bc2 flat
    din("identf", (128, 128))
    din("onesc", (128, 1))
    din("onesr", (1, 128))
    din("u0", (128, 6))

    out_t = nc.dram_tensor("out", [1, CLS], DT, kind="ExternalOutput")

    with ExitStack() as ctx:
        tc = ctx.enter_context(tile.TileContext(nc))
        wsm = ctx.enter_context(tc.tile_pool(name="wsm", bufs=2))
        wbg = ctx.enter_context(tc.tile_pool(name="wbg", bufs=4))
        vp = ctx.enter_context(tc.tile_pool(name="vp", bufs=2))
        pers = ctx.enter_context(tc.tile_pool(name="pers", bufs=1))
        wk = ctx.enter_context(tc.tile_pool(name="wk", bufs=1))
        ps_at = ctx.enter_context(tc.tile_pool(name="ps_at", bufs=1, space="PSUM"))
        ps_m = ctx.enter_context(tc.tile_pool(name="ps_m", bufs=1, space="PSUM"))
        ps_t = ctx.enter_context(tc.tile_pool(name="ps_t", bufs=1, space="PSUM"))

        psS = ps_t.tile([128, 128], DT)

        ident = pers.tile([128, 128], DT)
        nc.sync.dma_start(out=ident[:], in_=inp["identf"][:, :])
        onesc = pers.tile([128, 1], DT)
        nc.sync.dma_start(out=onesc[:], in_=inp["onesc"][:, :])
        onesr = pers.tile([1, 128], DT)
        nc.sync.dma_start(out=onesr[:], in_=inp["onesr"][:, :])
        epst = pers.tile([1, 1], DT)
        nc.vector.memset(epst[:], EPS)
        junk = pers.tile([1, 8], DT)
        nc.vector.memset(junk[:], 0.5)

        # canonical residual state + its square, feeding the stats matmul
        stat_in = pers.tile([128, 12], DT)
        u_cm = stat_in[:, 0:6]
        nc.sync.dma_start(out=stat_in[:, 0:6], in_=inp["u0"][:, :])

        def gelu_to(out_bf, in_f32, shp):
            if gelu_mode == 'hw':
                nc.scalar.activation(out=out_bf[:], in_=in_f32[:], func=ACT.Gelu)
                return
            y = wk.tile(shp, DT, tag="gely")
            nc.vector.tensor_mul(y[:], in_f32[:], in_f32[:])
            nc.vector.tensor_scalar(
                out=y[:], in0=y[:], scalar1=0.044715, scalar2=1.0,
                op0=OP.mult, op1=OP.add)
            nc.vector.tensor_mul(y[:], y[:], in_f32[:])
            nc.scalar.activation(out=y[:], in_=y[:], func=ACT.Tanh,
                                 scale=float(np.sqrt(2.0 / np.pi)))
            nc.vector.tensor_scalar(
                out=y[:], in0=y[:], scalar1=1.0, scalar2=0.5,
                op0=OP.add, op1=OP.mult)
            nc.vector.tensor_mul(y[:], in_f32[:], y[:])
            nc.vector.tensor_copy(out=out_bf[:], in_=y[:])

        def dummy(fn):
            # preload an activation table while other engines stream
            if gelu_mode != 'hw' and fn == ACT.Gelu:
                fn = ACT.Tanh
            nc.scalar.activation(out=junk[0:1, 4:8], in_=junk[0:1, 0:4], func=fn)

        def layer_norm_cm(x_cm, s_ap, b_ap, out_bf, tag):
            """x [128,6] cm -> (x-mu)*rstd*s+b, cast to bf16 [128,6]."""
            sq = stat_in[:, 6:12]
            nc.vector.tensor_mul(sq, x_cm, x_cm)
            pstat = psS[0:1, 0:12]
            nc.tensor.matmul(pstat, onesc[:], stat_in[:], start=True, stop=True)
            scal2 = wk.tile([1, 2], DT, tag=tag + "sc")
            scr6 = wk.tile([1, 6], DT, tag=tag + "s6")
            nc.vector.tensor_scalar(
                out=scr6[:], in0=psS[0:1, 0:6], scalar1=1.0 / E, scalar2=None,
                op0=OP.mult, op1=OP.add, accum_out=scal2[0:1, 0:1])
            msq = wk.tile([1, 1], DT, tag=tag + "ms")
            nc.vector.tensor_scalar(
                out=scr6[:], in0=psS[0:1, 6:12], scalar1=1.0 / E, scalar2=None,
                op0=OP.mult, op1=OP.add, accum_out=msq[:])
            mu2 = wk.tile([1, 1], DT, tag=tag + "m2")
            nc.vector.tensor_mul(mu2[:], scal2[0:1, 0:1], scal2[0:1, 0:1])
            var = wk.tile([1, 1], DT, tag=tag + "va")
            nc.vector.tensor_scalar(
                out=var[:], in0=msq[:], scalar1=1.0, scalar2=mu2[:],
                op0=OP.mult, op1=OP.subtract)
            sd = wk.tile([1, 1], DT, tag=tag + "sd")
            nc.scalar.activation(out=sd[:], in_=var[:], func=ACT.Sqrt, bias=epst[:])
            nc.vector.reciprocal(scal2[0:1, 1:2], sd[:])
            pbc = psS[:, 12:14]
            nc.tensor.matmul(pbc, onesr[:], scal2[:], start=True, stop=True)
            bcs = wk.tile([128, 2], DT, tag=tag + "bc")
            nc.vector.tensor_copy(out=bcs[:], in_=pbc)
            hf = wk.tile([128, 6], DT, tag=tag + "hf")
            nc.vector.tensor_scalar(
                out=hf[:], in0=x_cm, scalar1=bcs[:, 0:1], scalar2=bcs[:, 1:2],
                op0=OP.subtract, op1=OP.mult)
            nc.vector.tensor_mul(hf[:], hf[:], s_ap)
            nc.vector.tensor_add(hf[:], hf[:], b_ap)
            nc.vector.tensor_copy(out=out_bf[:], in_=hf[:])
            return hf

        def bcast1(val_ap, tag):
            """[1,1] -> [128,1] sbuf"""
            pb = psS[:, 14:15]
            nc.tensor.matmul(pb, onesr[:], val_ap, start=True, stop=True)
            sb = wk.tile([128, 1], DT, tag=tag)
            nc.vector.tensor_copy(out=sb[:], in_=pb)
            return sb

        def load_layer(l):
            pv = vp.tile([128, 84], DT, tag="pv", name=f"pv{l}_t")
            nc.sync.dma_start(out=pv[:], in_=inp[f"pv{l}"][:, :])
            wv_, wt_ = [], []
            for c in range(2):
                wvt = wsm.tile([128, 3 * E], BF, tag="wv", name=f"wv{c}_{l}_t")
                nc.sync.dma_start(out=wvt[:], in_=inp[f"wv{c}_{l}"][:, :])
                wv_.append(wvt)
                wtt = wsm.tile([128, 3 * E], BF, tag="wt", name=f"wt{c}_{l}_t")
                nc.sync.dma_start(out=wtt[:], in_=inp[f"wt{c}_{l}"][:, :])
                wt_.append(wtt)
            return wv_, wt_, pv

        nxt = load_layer(0)
        for l in range(L):
            wv_, wt_, pv = nxt
            w1c_ = []
            for c in range(6):
                wti = wbg.tile([128, HID], F8, tag="w1")
                nc.sync.dma_start(out=wti[:], in_=inp[f"w1{c}_{l}"][:, :])
                w1c_.append(wti)
            w2c_ = []
            for c in range(6):
                wti = wbg.tile([128, 4 * E], F8, tag="w2")
                nc.sync.dma_start(out=wti[:], in_=inp[f"w2{c}_{l}"][:, :])
                w2c_.append(wti)

            # ---- LN1 ----
            h_bf = wk.tile([128, 6], BF, tag="hbf")
            hf = layer_norm_cm(u_cm, pv[:, 0:6], pv[:, 6:12], h_bf, "l1")

            # ---- attn GEMVs: a = h@Wv, t = h@Wt; psum rows {0,32} x 384 ----
            psA = ps_at.tile([128, 384], DT, tag="pa")
            psB = ps_at.tile([128, 384], DT, tag="pb")
            for s in range(6):
                st, sp = (s == 0), (s == 5)
                lhs = h_bf[:, s:s + 1]
                c, sl = s // 3, s % 3
                for g in range(2):
                    nc.tensor.matmul(
                        psA[32 * g:32 * g + 1, 0:384], lhs,
                        wv_[c][:, sl * E + g * 384: sl * E + (g + 1) * 384],
                        start=st, stop=sp, tile_position=(0, 32 * g),
                        skip_group_check=True)
                for g in range(2):
                    nc.tensor.matmul(
                        psB[32 * g:32 * g + 1, 0:384], lhs,
                        wt_[c][:, sl * E + g * 384: sl * E + (g + 1) * 384],
                        start=st, stop=sp, tile_position=(0, 32 * g),
                        skip_group_check=True)

            if l + 1 < L:
                nxt = load_layer(l + 1)

            # ---- evac a,t rows (partitions 0,32) then per-128-block transposes ----
            at_sb = wk.tile([34, 768], DT, tag="atsb")
            nc.vector.tensor_copy(out=at_sb[0:1, 0:384], in_=psB[0:1, :])
            nc.vector.tensor_copy(out=at_sb[32:33, 0:384], in_=psB[32:33, :])
            nc.vector.tensor_copy(out=at_sb[0:1, 384:768], in_=psA[0:1, :])
            nc.vector.tensor_copy(out=at_sb[32:33, 384:768], in_=psA[32:33, :])
            # t_cm cols 16+j, a_cm cols 22+j; seg j = 3r+c
            for r in range(2):
                idr = ident[32 * r:32 * r + 1, 32 * r:32 * r + 1]
                for c in range(3):
                    nc.tensor.transpose(
                        psS[:, 16 + 3 * r + c:17 + 3 * r + c],
                        at_sb[32 * r:32 * r + 1, 128 * c:128 * c + 128], idr)
                    nc.tensor.transpose(
                        psS[:, 22 + 3 * r + c:23 + 3 * r + c],
                        at_sb[32 * r:32 * r + 1, 384 + 128 * c:384 + 128 * c + 128], idr)
            # sval = h . t
            scr = wk.tile([128, 6], DT, tag="scr6b")
            nc.vector.tensor_mul(scr[:], hf[:], psS[:, 16:22])
            pdd = psS[0:1, 28:34]
            nc.tensor.matmul(pdd, onesc[:], scr[:], start=True, stop=True)
            s6 = wk.tile([1, 6], DT, tag="sv6")
            sval = wk.tile([1, 1], DT, tag="sval")
            nc.vector.tensor_scalar(
                out=s6[:], in0=psS[0:1, 28:34], scalar1=INV_SQRT_E, scalar2=None,
                op0=OP.mult, op1=OP.add, accum_out=sval[:])
            c0 = wk.tile([1, 1], DT, tag="c0")
            nc.vector.tensor_scalar(
                out=c0[:], in0=sval[:], scalar1=1.0, scalar2=None, op0=OP.add)
            c0b = bcast1(c0[:], "c0b")
            # u' = h + a*c0   (into the canonical u slot)
            nc.vector.tensor_scalar(
                out=u_cm, in0=psS[:, 22:28], scalar1=c0b[:, 0:1], scalar2=None,
                op0=OP.mult)
            nc.vector.tensor_add(u_cm, u_cm, hf[:])

            # ---- LN2 ----
            h2_bf = wk.tile([128, 6], BF, tag="h2bf")
            layer_norm_cm(u_cm, pv[:, 12:18], pv[:, 18:24], h2_bf, "l2")
            dummy(ACT.Gelu)   # table load hidden under W1 GEMV

            # ---- W1 GEMV: 6 passes x 6 chunks of 512 ----
            psC = ps_m.tile([128, 512], DT, tag="mC")
            psD = ps_m.tile([128, 512], DT, tag="mD")
            for s in range(6):
                st, sp = (s == 0), (s == 5)
                lhs = h2_bf[:, s:s + 1]
                for nt in range(6):
                    pt, row = (psC, nt) if nt < 4 else (psD, nt - 4)
                    nc.tensor.matmul(
                        pt[32 * row:32 * row + 1, 0:512], lhs,
                        w1c_[s][:, nt * 512: nt * 512 + 512],
                        start=st, stop=sp, tile_position=(0, 32 * row),
                        skip_group_check=True)
            # evac rows -> [6,512], transpose to cm [128,24]
            m1r = wk.tile([128, 512], DT, tag="m1r")
            for r in range(4):
                nc.vector.tensor_copy(
                    out=m1r[32 * r:32 * r + 1, :], in_=psC[32 * r:32 * r + 1, :])
            m1r2 = wk.tile([34, 512], DT, tag="m1r2")
            nc.vector.tensor_copy(out=m1r2[0:1, :], in_=psD[0:1, :])
            nc.vector.tensor_copy(out=m1r2[32:33, :], in_=psD[32:33, :])
            gps = psS[:, 34:58]
            for nt in range(6):   # m1 chunk nt -> cm cols 4nt..4nt+3
                if nt < 4:
                    srcr, base = m1r, 32 * nt
                else:
                    srcr, base = m1r2, 32 * (nt - 4)
                idr = ident[base:base + 1, base:base + 1]
                for c in range(4):
                    nc.tensor.transpose(
                        psS[:, 34 + 4 * nt + c:35 + 4 * nt + c],
                        srcr[base:base + 1, 128 * c:128 * c + 128], idr)
            # m1 = m1q*s1 + b1 ; gelu -> bf16
            gf = wk.tile([128, 24], DT, tag="gf")
            nc.vector.tensor_mul(gf[:], pv[:, 36:60], gps)
            nc.vector.tensor_add(gf[:], gf[:], pv[:, 60:84])
            g_bf = wk.tile([128, 24], BF, tag="gbf")
            gelu_to(g_bf, gf, [128, 24])
            dummy(ACT.Sqrt)   # table load hidden under W2 GEMV

            # ---- W2 GEMV: 24 passes x 2 chunks of 384 ----
            psE = ps_m.tile([128, 384], DT, tag="mE")
            for s in range(24):
                st, sp = (s == 0), (s == 23)
                lhs = g_bf[:, s:s + 1]
                wsrc = w2c_[s // 4]
                sl = s % 4
                for g in range(2):
                    nc.tensor.matmul(
                        psE[32 * g:32 * g + 1, 0:384], lhs,
                        wsrc[:, sl * E + g * 384: sl * E + (g + 1) * 384],
                        start=st, stop=sp, tile_position=(0, 32 * g),
                        skip_group_check=True)
            m2r = wk.tile([34, 384], DT, tag="m2r")
            nc.vector.tensor_copy(out=m2r[0:1, :], in_=psE[0:1, :])
            nc.vector.tensor_copy(out=m2r[32:33, :], in_=psE[32:33, :])
            pu2 = psS[:, 58:64]
            for r in range(2):
                idr = ident[32 * r:32 * r + 1, 32 * r:32 * r + 1]
                for c in range(3):
                    nc.tensor.transpose(
                        psS[:, 58 + 3 * r + c:59 + 3 * r + c],
                        m2r[32 * r:32 * r + 1, 128 * c:128 * c + 128], idr)
            # u'' = u' + m2q*s2 + b2
            d6 = wk.tile([128, 6], DT, tag="d6")
            nc.vector.tensor_mul(d6[:], pv[:, 24:30], pu2)
            nc.vector.tensor_add(d6[:], d6[:], pv[:, 30:36])
            nc.vector.tensor_add(u_cm, u_cm, d6[:])

        # ---- classifier ----
        fcm = vp.tile([128, 36], DT, tag="pv")
        nc.sync.dma_start(out=fcm[:], in_=inp["fcm"][:, :])
        fb = pers.tile([1, CLS], DT)
        nc.sync.dma_start(out=fb[:], in_=inp["fb"][:, :])
        wc1c_ = []
        for c in range(6):
            wti = wbg.tile([128, HID], BF, tag="wc1")
            nc.sync.dma_start(out=wti[:], in_=inp[f"wc1{c}"][:, :])
            wc1c_.append(wti)

        cls_bf = wk.tile([128, 6], BF, tag="hbf")
        layer_norm_cm(u_cm, fcm[:, 0:6], fcm[:, 6:12], cls_bf, "lf")
        dummy(ACT.Gelu)

        psC = ps_m.tile([128, 512], DT, tag="mC")
        psD = ps_m.tile([128, 512], DT, tag="mD")
        for s in range(6):
            st, sp = (s == 0), (s == 5)
            lhs = cls_bf[:, s:s + 1]
            for nt in range(6):
                pt, row = (psC, nt) if nt < 4 else (psD, nt - 4)
                nc.tensor.matmul(
                    pt[32 * row:32 * row + 1, 0:512], lhs,
                    wc1c_[s][:, nt * 512: nt * 512 + 512],
                    start=st, stop=sp, tile_position=(0, 32 * row),
                    skip_group_check=True)
        m1r = wk.tile([128, 512], DT, tag="m1r")
        for r in range(4):
            nc.vector.tensor_copy(
                out=m1r[32 * r:32 * r + 1, :], in_=psC[32 * r:32 * r + 1, :])
        m1r2 = wk.tile([34, 512], DT, tag="m1r2")
        nc.vector.tensor_copy(out=m1r2[0:1, :], in_=psD[0:1, :])
        nc.vector.tensor_copy(out=m1r2[32:33, :], in_=psD[32:33, :])
        gps = psS[:, 34:58]
        for nt in range(6):
            if nt < 4:
                srcr, base = m1r, 32 * nt
            else:
                srcr, base = m1r2, 32 * (nt - 4)
            idr = ident[base:base + 1, base:base + 1]
            for c in range(4):
                nc.tensor.transpose(
                    psS[:, 34 + 4 * nt + c:35 + 4 * nt + c],
                    srcr[base:base + 1, 128 * c:128 * c + 128], idr)
        gf = wk.tile([128, 24], DT, tag="gf")
        nc.vector.tensor_add(gf[:], fcm[:, 12:36], gps)
        gc_bf = wk.tile([128, 24], BF, tag="gbf")
        gelu_to(gc_bf, gf, [128, 24])

        wc2 = []
        for c in range(8):
            w = wbg.tile([128, 3 * CLS], BF, tag="wc2")
            nc.sync.dma_start(out=w[:], in_=inp[f"wc2{c}"][:, :])
            wc2.append(w)
        psF = ps_m.tile([128, 512], DT, tag="mF")
        for s in range(24):
            st, sp = (s == 0), (s == 23)
            lhs = gc_bf[:, s:s + 1]
            wsrc = wc2[s // 3]
            sl = s % 3
            for g in range(2):
                nc.tensor.matmul(
                    psF[32 * g:32 * g + 1, 0:500], lhs,
                    wsrc[:, sl * CLS + g * 500: sl * CLS + (g + 1) * 500],
                    start=st, stop=sp, tile_position=(0, 32 * g),
                    skip_group_check=True)
        lg = wk.tile([1, CLS], DT, tag="lg")
        nc.vector.tensor_copy(out=lg[0:1, 0:500], in_=psF[0:1, 0:500])
        nc.vector.tensor_copy(out=lg[0:1, 500:1000], in_=psF[32:33, 0:500])
        nc.vector.tensor_add(lg[:], lg[:], fb[:])

        # log_softmax
        mx = wk.tile([1, 1], DT, tag="mx")
        nc.vector.reduce_max(mx[:], lg[:], axis=AX.X)
        sh = wk.tile([1, CLS], DT, tag="sh")
        nc.vector.tensor_scalar(
            out=sh[:], in0=lg[:], scalar1=mx[:], scalar2=None, op0=OP.subtract)
        se = wk.tile([1, 1], DT, tag="se")
        nc.scalar.activation(out=lg[:], in_=sh[:], func=ACT.Exp, accum_out=se[:])
        lse = wk.tile([1, 1], DT, tag="lse")
        nc.scalar.activation(out=lse[:], in_=se[:], func=ACT.Ln)
        nc.vector.tensor_scalar(
            out=sh[:], in0=sh[:], scalar1=lse[:], scalar2=None, op0=OP.subtract)
        nc.sync.dma_start(out=out_t[:, :], in_=sh[:])

    nc.compile()
    return nc


def _cm(v, nseg):
    """flat [-1] -> [128, nseg] with cm[p, s] = v[128s + p]"""
    return np.ascontiguousarray(np.asarray(v, np.float32).reshape(nseg, 128).T)


def prep_inputs(inputs):
    f32 = lambda x: np.ascontiguousarray(np.asarray(x, dtype=np.float32))
    bf = lambda x: np.ascontiguousarray(
        np.asarray(x, dtype=np.float32).astype(ml_dtypes.bfloat16))
    m = {}
    Wv, Wt = inputs["Wv"], inputs["Wtheta"]
    W1, W2 = inputs["W1"], inputs["W2"]
    for l in range(L):
        wv = np.asarray(Wv[l]).reshape(6, 128, E).transpose(1, 0, 2)
        wt = np.asarray(Wt[l]).reshape(6, 128, E).transpose(1, 0, 2)
        for c in range(2):
            m[f"wv{c}_{l}"] = bf(wv[:, 3 * c:3 * c + 3].reshape(128, 3 * E))
            m[f"wt{c}_{l}"] = bf(wt[:, 3 * c:3 * c + 3].reshape(128, 3 * E))
        w1 = np.asarray(W1[l], np.float32)              # [E, HID]
        s1 = np.abs(w1).max(axis=0) / F8LIM             # [HID]
        w1q = (w1 / s1).reshape(6, 128, HID).transpose(1, 0, 2)
        for c in range(6):
            m[f"w1{c}_{l}"] = np.ascontiguousarray(
                w1q[:, c].reshape(128, HID).astype(ml_dtypes.float8_e3m4))
        w2 = np.asarray(W2[l], np.float32)              # [HID, E]
        s2 = np.abs(w2).max(axis=0) / F8LIM             # [E]
        w2q = (w2 / s2).reshape(24, 128, E).transpose(1, 0, 2)
        for c in range(6):
            m[f"w2{c}_{l}"] = np.ascontiguousarray(
                w2q[:, 4 * c:4 * c + 4].reshape(128, 4 * E).astype(
                    ml_dtypes.float8_e3m4))
        pv = np.concatenate([
            _cm(inputs["ln1_s"][l], 6), _cm(inputs["ln1_b"][l], 6),
            _cm(inputs["ln2_s"][l], 6), _cm(inputs["ln2_b"][l], 6),
            _cm(s2, 6), _cm(inputs["b2"][l], 6),
            _cm(s1, 24), _cm(inputs["b1"][l], 24)], axis=1)
        m[f"pv{l}"] = f32(pv)
    wc1 = np.asarray(inputs["Wc1"]).reshape(6, 128, HID).transpose(1, 0, 2)
    for c in range(6):
        m[f"wc1{c}"] = bf(wc1[:, c].reshape(128, HID))
    wc2 = np.asarray(inputs["Wc2"]).reshape(24, 128, CLS).transpose(1, 0, 2)
    for c in range(8):
        m[f"wc2{c}"] = bf(wc2[:, 3 * c:3 * c + 3].reshape(128, 3 * CLS))
    m["fcm"] = f32(np.concatenate([
        _cm(inputs["lnf_s"], 6), _cm(inputs["lnf_b"], 6),
        _cm(inputs["bc1"], 24)], axis=1))
    m["fb"] = f32(np.asarray(inputs["bc2"]).reshape(1, CLS))
    m["identf"] = np.eye(128, dtype=np.float32)
    m["onesc"] = np.ones((128, 1), np.float32)
    m["onesr"] = np.ones((1, 128), np.float32)
    u0 = np.asarray(inputs["class_token"]).reshape(E) + \
        np.asarray(inputs["pos"]).reshape(-1, E)[-1]
    m["u0"] = _cm(u0, 6)
    return m


_CACHED = {}


def kernel(**inputs) -> np.ndarray:
    b = int(np.asarray(inputs["x"]).shape[0])
    in_map = prep_inputs(inputs)
    if "nc" not in _CACHED:
        _CACHED["nc"] = build_program()
    nc = _CACHED["nc"]
    r = run_bass_kernel_spmd(nc, [in_map], core_ids=[0])
    out = np.asarray(r.results[0]["out"]).reshape(1, CLS)
    return np.ascontiguousarray(np.broadcast_to(out, (b, CLS)).astype(np.float32))


# revision 4
# speedup vs baseline: 2.4775x; 1.2725x over previous
"""Trainium2 Bass kernel for nn_EnoughViT_63282048139394 — v2.

Same math reduction as v1 (single-token chain, batch-broadcast output), but:
  - activations live in column-major [128, segs] layout; LayerNorm stats via a
    ones-matmul partition reduce; affine applied with per-partition scalars
    (no 1-lane flat vector ops)
  - layout changes (flat<->cm) via PE transpose-mode, not K=1 matmul chains
  - W1/W2 stored as per-column-scaled float8_e3m4 (scales folded back on the
    CM side after the GEMV), attention + classifier weights bf16
  - activation-table thrash hidden by dummy Gelu/Sqrt ops issued while the
    tensor engine streams the next GEMV
"""

import numpy as np
import ml_dtypes
from contextlib import ExitStack

import concourse.bass as bass
import concourse.tile as tile
from concourse import bacc, mybir
from concourse.bass_utils import run_bass_kernel_spmd

E = 768
HID = 3072
CLS = 1000
L = 12
EPS = 1e-5
INV_SQRT_E = 1.0 / float(np.sqrt(768.0))
DT = mybir.dt.float32
BF = mybir.dt.bfloat16
F8 = mybir.dt.float8e3
AX = mybir.AxisListType
OP = mybir.AluOpType
ACT = mybir.ActivationFunctionType
F8LIM = 12.0


def build_program(gelu_mode='hw'):
    nc = bacc.Bacc()
    inp = {}

    def din(name, shape, dt=DT):
        t = nc.dram_tensor(name, list(shape), dt, kind="ExternalInput")
        inp[name] = t
        return t

    for l in range(L):
        for c in range(2):
            din(f"wv{c}_{l}", (128, 3 * E), BF)
            din(f"wt{c}_{l}", (128, 3 * E), BF)
        for c in range(6):
            din(f"w1{c}_{l}", (128, HID), F8)
        for c in range(6):
            din(f"w2{c}_{l}", (128, 4 * E), F8)
        din(f"pv{l}", (128, 84))   # ln1_s|ln1_b|ln2_s|ln2_b|s2|b2 (6 each), s1(24), b1(24)
    for c in range(6):
        din(f"wc1{c}", (128, HID), BF)
    for c in range(8):
        din(f"wc2{c}", (128, 3 * CLS), BF)
    din("fcm", (128, 36))          # lnf_s, lnf_b cm + bc1 cm
    din("fb", (1, CLS))            # bc2 flat
    din("identf", (128, 128))
    din("onesc", (128, 1))
    din("onesr", (1, 128))
    din("u0", (128, 6))

    out_t = nc.dram_tensor("out", [1, CLS], DT, kind="ExternalOutput")

    with ExitStack() as ctx:
        tc = ctx.enter_context(tile.TileContext(nc))
        wsm = ctx.enter_context(tc.tile_pool(name="wsm", bufs=2))
        wbg = ctx.enter_context(tc.tile_pool(name="wbg", bufs=8))
        vp = ctx.enter_context(tc.tile_pool(name="vp", bufs=2))
        pers = ctx.enter_context(tc.tile_pool(name="pers", bufs=1))
        wk = ctx.enter_context(tc.tile_pool(name="wk", bufs=1))
        ps_at = ctx.enter_context(tc.tile_pool(name="ps_at", bufs=1, space="PSUM"))
        ps_m = ctx.enter_context(tc.tile_pool(name="ps_m", bufs=1, space="PSUM"))
        ps_t = ctx.enter_context(tc.tile_pool(name="ps_t", bufs=1, space="PSUM"))

        psS = ps_t.tile([128, 128], DT)

        ident = pers.tile([128, 128], DT)
        nc.sync.dma_start(out=ident[:], in_=inp["identf"][:, :])
        onesc = pers.tile([128, 1], DT)
        nc.sync.dma_start(out=onesc[:], in_=inp["onesc"][:, :])
        onesr = pers.tile([1, 128], DT)
        nc.sync.dma_start(out=onesr[:], in_=inp["onesr"][:, :])
        epst = pers.tile([1, 1], DT)
        nc.vector.memset(epst[:], EPS)
        junk = pers.tile([1, 8], DT)
        nc.vector.memset(junk[:], 0.5)

        # canonical residual state + its square, feeding the stats matmul
        stat_in = pers.tile([128, 12], DT)
        u_cm = stat_in[:, 0:6]
        nc.sync.dma_start(out=stat_in[:, 0:6], in_=inp["u0"][:, :])

        def gelu_to(out_bf, in_f32, shp):
            if gelu_mode == 'hw':
                nc.scalar.activation(out=out_bf[:], in_=in_f32[:], func=ACT.Gelu)
                return
            y = wk.tile(shp, DT, tag="gely")
            nc.vector.tensor_mul(y[:], in_f32[:], in_f32[:])
            nc.vector.tensor_scalar(
                out=y[:], in0=y[:], scalar1=0.044715, scalar2=1.0,
                op0=OP.mult, op1=OP.add)
            nc.vector.tensor_mul(y[:], y[:], in_f32[:])
            nc.scalar.activation(out=y[:], in_=y[:], func=ACT.Tanh,
                                 scale=float(np.sqrt(2.0 / np.pi)))
            nc.vector.tensor_scalar(
                out=y[:], in0=y[:], scalar1=1.0, scalar2=0.5,
                op0=OP.add, op1=OP.mult)
            nc.vector.tensor_mul(y[:], in_f32[:], y[:])
            nc.vector.tensor_copy(out=out_bf[:], in_=y[:])

        def dummy(fn):
            # preload an activation table while other engines stream
            if gelu_mode != 'hw' and fn == ACT.Gelu:
                fn = ACT.Tanh
            nc.scalar.activation(out=junk[0:1, 4:8], in_=junk[0:1, 0:4], func=fn)

        def layer_norm_cm(x_cm, s_ap, b_ap, out_bf, tag):
            """x [128,6] cm -> (x-mu)*rstd*s+b, cast to bf16 [128,6]."""
            sq = stat_in[:, 6:12]
            nc.vector.tensor_mul(sq, x_cm, x_cm)
            pstat = psS[0:1, 0:12]
            nc.tensor.matmul(pstat, onesc[:], stat_in[:], start=True, stop=True)
            scal2 = wk.tile([1, 2], DT, tag=tag + "sc")
            scr6 = wk.tile([1, 6], DT, tag=tag + "s6")
            nc.vector.tensor_scalar(
                out=scr6[:], in0=psS[0:1, 0:6], scalar1=1.0 / E, scalar2=None,
                op0=OP.mult, op1=OP.add, accum_out=scal2[0:1, 0:1])
            msq = wk.tile([1, 1], DT, tag=tag + "ms")
            nc.vector.tensor_scalar(
                out=scr6[:], in0=psS[0:1, 6:12], scalar1=1.0 / E, scalar2=None,
                op0=OP.mult, op1=OP.add, accum_out=msq[:])
            mu2 = wk.tile([1, 1], DT, tag=tag + "m2")
            nc.vector.tensor_mul(mu2[:], scal2[0:1, 0:1], scal2[0:1, 0:1])
            var = wk.tile([1, 1], DT, tag=tag + "va")
            nc.vector.tensor_scalar(
                out=var[:], in0=msq[:], scalar1=1.0, scalar2=mu2[:],
                op0=OP.mult, op1=OP.subtract)
            sd = wk.tile([1, 1], DT, tag=tag + "sd")
            nc.scalar.activation(out=sd[:], in_=var[:], func=ACT.Sqrt, bias=epst[:])
            nc.vector.reciprocal(scal2[0:1, 1:2], sd[:])
            pbc = psS[:, 12:14]
            nc.tensor.matmul(pbc, onesr[:], scal2[:], start=True, stop=True)
            bcs = wk.tile([128, 2], DT, tag=tag + "bc")
            nc.vector.tensor_copy(out=bcs[:], in_=pbc)
            hf = wk.tile([128, 6], DT, tag=tag + "hf")
            nc.vector.tensor_scalar(
                out=hf[:], in0=x_cm, scalar1=bcs[:, 0:1], scalar2=bcs[:, 1:2],
                op0=OP.subtract, op1=OP.mult)
            nc.vector.tensor_mul(hf[:], hf[:], s_ap)
            nc.vector.tensor_add(hf[:], hf[:], b_ap)
            nc.vector.tensor_copy(out=out_bf[:], in_=hf[:])
            return hf

        def bcast1(val_ap, tag):
            """[1,1] -> [128,1] sbuf"""
            pb = psS[:, 14:15]
            nc.tensor.matmul(pb, onesr[:], val_ap, start=True, stop=True)
            sb = wk.tile([128, 1], DT, tag=tag)
            nc.vector.tensor_copy(out=sb[:], in_=pb)
            return sb

        def load_layer(l):
            pv = vp.tile([128, 84], DT, tag="pv", name=f"pv{l}_t")
            nc.sync.dma_start(out=pv[:], in_=inp[f"pv{l}"][:, :])
            wv_, wt_ = [], []
            for c in range(2):
                wvt = wsm.tile([128, 3 * E], BF, tag="wv", name=f"wv{c}_{l}_t")
                nc.sync.dma_start(out=wvt[:], in_=inp[f"wv{c}_{l}"][:, :])
                wv_.append(wvt)
                wtt = wsm.tile([128, 3 * E], BF, tag="wt", name=f"wt{c}_{l}_t")
                nc.sync.dma_start(out=wtt[:], in_=inp[f"wt{c}_{l}"][:, :])
                wt_.append(wtt)
            return wv_, wt_, pv

        nxt = load_layer(0)
        for l in range(L):
            wv_, wt_, pv = nxt
            w1c_ = []
            for c in range(6):
                wti = wbg.tile([128, HID], F8, tag="w1")
                nc.sync.dma_start(out=wti[:], in_=inp[f"w1{c}_{l}"][:, :])
                w1c_.append(wti)
            w2c_ = []
            for c in range(6):
                wti = wbg.tile([128, 4 * E], F8, tag="w2")
                nc.sync.dma_start(out=wti[:], in_=inp[f"w2{c}_{l}"][:, :])
                w2c_.append(wti)

            # ---- LN1 ----
            h_bf = wk.tile([128, 6], BF, tag="hbf")
            hf = layer_norm_cm(u_cm, pv[:, 0:6], pv[:, 6:12], h_bf, "l1")

            # ---- attn GEMVs: a = h@Wv, t = h@Wt; psum rows {0,32} x 384 ----
            psA = ps_at.tile([128, 384], DT, tag="pa")
            psB = ps_at.tile([128, 384], DT, tag="pb")
            for s in range(6):
                st, sp = (s == 0), (s == 5)
                lhs = h_bf[:, s:s + 1]
                c, sl = s // 3, s % 3
                for g in range(2):
                    nc.tensor.matmul(
                        psA[32 * g:32 * g + 1, 0:384], lhs,
                        wv_[c][:, sl * E + g * 384: sl * E + (g + 1) * 384],
                        start=st, stop=sp, tile_position=(0, 32 * g),
                        skip_group_check=True)
                for g in range(2):
                    nc.tensor.matmul(
                        psB[32 * g:32 * g + 1, 0:384], lhs,
                        wt_[c][:, sl * E + g * 384: sl * E + (g + 1) * 384],
                        start=st, stop=sp, tile_position=(0, 32 * g),
                        skip_group_check=True)

            if l + 1 < L:
                nxt = load_layer(l + 1)

            # ---- evac a,t rows (partitions 0,32) then per-128-block transposes ----
            at_sb = wk.tile([34, 768], DT, tag="atsb")
            nc.vector.tensor_copy(out=at_sb[0:1, 0:384], in_=psB[0:1, :])
            nc.vector.tensor_copy(out=at_sb[32:33, 0:384], in_=psB[32:33, :])
            nc.vector.tensor_copy(out=at_sb[0:1, 384:768], in_=psA[0:1, :])
            nc.vector.tensor_copy(out=at_sb[32:33, 384:768], in_=psA[32:33, :])
            # t_cm cols 16+j, a_cm cols 22+j; seg j = 3r+c
            for r in range(2):
                idr = ident[32 * r:32 * r + 1, 32 * r:32 * r + 1]
                for c in range(3):
                    nc.tensor.transpose(
                        psS[:, 16 + 3 * r + c:17 + 3 * r + c],
                        at_sb[32 * r:32 * r + 1, 128 * c:128 * c + 128], idr)
                    nc.tensor.transpose(
                        psS[:, 22 + 3 * r + c:23 + 3 * r + c],
                        at_sb[32 * r:32 * r + 1, 384 + 128 * c:384 + 128 * c + 128], idr)
            # sval = h . t
            scr = wk.tile([128, 6], DT, tag="scr6b")
            nc.vector.tensor_mul(scr[:], hf[:], psS[:, 16:22])
            pdd = psS[0:1, 28:34]
            nc.tensor.matmul(pdd, onesc[:], scr[:], start=True, stop=True)
            s6 = wk.tile([1, 6], DT, tag="sv6")
            sval = wk.tile([1, 1], DT, tag="sval")
            nc.vector.tensor_scalar(
                out=s6[:], in0=psS[0:1, 28:34], scalar1=INV_SQRT_E, scalar2=None,
                op0=OP.mult, op1=OP.add, accum_out=sval[:])
            c0 = wk.tile([1, 1], DT, tag="c0")
            nc.vector.tensor_scalar(
                out=c0[:], in0=sval[:], scalar1=1.0, scalar2=None, op0=OP.add)
            c0b = bcast1(c0[:], "c0b")
            # u' = h + a*c0   (into the canonical u slot)
            nc.vector.tensor_scalar(
                out=u_cm, in0=psS[:, 22:28], scalar1=c0b[:, 0:1], scalar2=None,
                op0=OP.mult)
            nc.vector.tensor_add(u_cm, u_cm, hf[:])

            # ---- LN2 ----
            h2_bf = wk.tile([128, 6], BF, tag="h2bf")
            layer_norm_cm(u_cm, pv[:, 12:18], pv[:, 18:24], h2_bf, "l2")
            dummy(ACT.Gelu)   # table load hidden under W1 GEMV

            # ---- W1 GEMV: 6 passes x 6 chunks of 512 ----
            psC = ps_m.tile([128, 512], DT, tag="mC")
            psD = ps_m.tile([128, 512], DT, tag="mD")
            for s in range(6):
                st, sp = (s == 0), (s == 5)
                lhs = h2_bf[:, s:s + 1]
                for nt in range(6):
                    pt, row = (psC, nt) if nt < 4 else (psD, nt - 4)
                    nc.tensor.matmul(
                        pt[32 * row:32 * row + 1, 0:512], lhs,
                        w1c_[s][:, nt * 512: nt * 512 + 512],
                        start=st, stop=sp, tile_position=(0, 32 * row),
                        skip_group_check=True)
            # evac rows -> [6,512], transpose to cm [128,24]
            m1r = wk.tile([128, 512], DT, tag="m1r")
            for r in range(4):
                nc.vector.tensor_copy(
                    out=m1r[32 * r:32 * r + 1, :], in_=psC[32 * r:32 * r + 1, :])
            m1r2 = wk.tile([34, 512], DT, tag="m1r2")
            nc.vector.tensor_copy(out=m1r2[0:1, :], in_=psD[0:1, :])
            nc.vector.tensor_copy(out=m1r2[32:33, :], in_=psD[32:33, :])
            gps = psS[:, 34:58]
            for nt in range(6):   # m1 chunk nt -> cm cols 4nt..4nt+3
                if nt < 4:
                    srcr, base = m1r, 32 * nt
                else:
                    srcr, base = m1r2, 32 * (nt - 4)
                idr = ident[base:base + 1, base:base + 1]
                for c in range(4):
                    nc.tensor.transpose(
                        psS[:, 34 + 4 * nt + c:35 + 4 * nt + c],
                        srcr[base:base + 1, 128 * c:128 * c + 128], idr)
            # m1 = m1q*s1 + b1 ; gelu -> bf16
            gf = wk.tile([128, 24], DT, tag="gf")
            nc.vector.tensor_mul(gf[:], pv[:, 36:60], gps)
            nc.vector.tensor_add(gf[:], gf[:], pv[:, 60:84])
            g_bf = wk.tile([128, 24], BF, tag="gbf")
            gelu_to(g_bf, gf, [128, 24])
            dummy(ACT.Sqrt)   # table load hidden under W2 GEMV

            # ---- W2 GEMV: 24 passes x 2 chunks of 384 ----
            psE = ps_m.tile([128, 384], DT, tag="mE")
            for s in range(24):
                st, sp = (s == 0), (s == 23)
                lhs = g_bf[:, s:s + 1]
                wsrc = w2c_[s // 4]
                sl = s % 4
                for g in range(2):
                    nc.tensor.matmul(
                        psE[32 * g:32 * g + 1, 0:384], lhs,
                        wsrc[:, sl * E + g * 384: sl * E + (g + 1) * 384],
                        start=st, stop=sp, tile_position=(0, 32 * g),
                        skip_group_check=True)
            m2r = wk.tile([34, 384], DT, tag="m2r")
            nc.vector.tensor_copy(out=m2r[0:1, :], in_=psE[0:1, :])
            nc.vector.tensor_copy(out=m2r[32:33, :], in_=psE[32:33, :])
            pu2 = psS[:, 58:64]
            for r in range(2):
                idr = ident[32 * r:32 * r + 1, 32 * r:32 * r + 1]
                for c in range(3):
                    nc.tensor.transpose(
                        psS[:, 58 + 3 * r + c:59 + 3 * r + c],
                        m2r[32 * r:32 * r + 1, 128 * c:128 * c + 128], idr)
            # u'' = u' + m2q*s2 + b2
            d6 = wk.tile([128, 6], DT, tag="d6")
            nc.vector.tensor_mul(d6[:], pv[:, 24:30], pu2)
            nc.vector.tensor_add(d6[:], d6[:], pv[:, 30:36])
            nc.vector.tensor_add(u_cm, u_cm, d6[:])

        # ---- classifier ----
        fcm = vp.tile([128, 36], DT, tag="pv")
        nc.sync.dma_start(out=fcm[:], in_=inp["fcm"][:, :])
        fb = pers.tile([1, CLS], DT)
        nc.sync.dma_start(out=fb[:], in_=inp["fb"][:, :])
        wc1c_ = []
        for c in range(6):
            wti = wbg.tile([128, HID], BF, tag="wc1")
            nc.sync.dma_start(out=wti[:], in_=inp[f"wc1{c}"][:, :])
            wc1c_.append(wti)

        cls_bf = wk.tile([128, 6], BF, tag="hbf")
        layer_norm_cm(u_cm, fcm[:, 0:6], fcm[:, 6:12], cls_bf, "lf")
        dummy(ACT.Gelu)

        psC = ps_m.tile([128, 512], DT, tag="mC")
        psD = ps_m.tile([128, 512], DT, tag="mD")
        for s in range(6):
            st, sp = (s == 0), (s == 5)
            lhs = cls_bf[:, s:s + 1]
            for nt in range(6):
                pt, row = (psC, nt) if nt < 4 else (psD, nt - 4)
                nc.tensor.matmul(
                    pt[32 * row:32 * row + 1, 0:512], lhs,
                    wc1c_[s][:, nt * 512: nt * 512 + 512],
                    start=st, stop=sp, tile_position=(0, 32 * row),
                    skip_group_check=True)
        m1r = wk.tile([128, 512], DT, tag="m1r")
        for r in range(4):
            nc.vector.tensor_copy(
                out=m1r[32 * r:32 * r + 1, :], in_=psC[32 * r:32 * r + 1, :])
        m1r2 = wk.tile([34, 512], DT, tag="m1r2")
        nc.vector.tensor_copy(out=m1r2[0:1, :], in_=psD[0:1, :])
        nc.vector.tensor_copy(out=m1r2[32:33, :], in_=psD[32:33, :])
        gps = psS[:, 34:58]
        for nt in range(6):
            if nt < 4:
                srcr, base = m1r, 32 * nt
            else:
                srcr, base = m1r2, 32 * (nt - 4)
            idr = ident[base:base + 1, base:base + 1]
            for c in range(4):
                nc.tensor.transpose(
                    psS[:, 34 + 4 * nt + c:35 + 4 * nt + c],
                    srcr[base:base + 1, 128 * c:128 * c + 128], idr)
        gf = wk.tile([128, 24], DT, tag="gf")
        nc.vector.tensor_add(gf[:], fcm[:, 12:36], gps)
        gc_bf = wk.tile([128, 24], BF, tag="gbf")
        gelu_to(gc_bf, gf, [128, 24])

        wc2 = []
        for c in range(8):
            w = wbg.tile([128, 3 * CLS], BF, tag="wc2")
            nc.sync.dma_start(out=w[:], in_=inp[f"wc2{c}"][:, :])
            wc2.append(w)
        psF = ps_m.tile([128, 512], DT, tag="mF")
        for s in range(24):
            st, sp = (s == 0), (s == 23)
            lhs = gc_bf[:, s:s + 1]
            wsrc = wc2[s // 3]
            sl = s % 3
            for g in range(2):
                nc.tensor.matmul(
                    psF[32 * g:32 * g + 1, 0:500], lhs,
                    wsrc[:, sl * CLS + g * 500: sl * CLS + (g + 1) * 500],
                    start=st, stop=sp, tile_position=(0, 32 * g),
                    skip_group_check=True)
        lg = wk.tile([1, CLS], DT, tag="lg")
        nc.vector.tensor_copy(out=lg[0:1, 0:500], in_=psF[0:1, 0:500])
        nc.vector.tensor_copy(out=lg[0:1, 500:1000], in_=psF[32:33, 0:500])
        nc.vector.tensor_add(lg[:], lg[:], fb[:])

        # log_softmax
        mx = wk.tile([1, 1], DT, tag="mx")
        nc.vector.reduce_max(mx[:], lg[:], axis=AX.X)
        sh = wk.tile([1, CLS], DT, tag="sh")
        nc.vector.tensor_scalar(
            out=sh[:], in0=lg[:], scalar1=mx[:], scalar2=None, op0=OP.subtract)
        se = wk.tile([1, 1], DT, tag="se")
        nc.scalar.activation(out=lg[:], in_=sh[:], func=ACT.Exp, accum_out=se[:])
        lse = wk.tile([1, 1], DT, tag="lse")
        nc.scalar.activation(out=lse[:], in_=se[:], func=ACT.Ln)
        nc.vector.tensor_scalar(
            out=sh[:], in0=sh[:], scalar1=lse[:], scalar2=None, op0=OP.subtract)
        nc.sync.dma_start(out=out_t[:, :], in_=sh[:])

    nc.compile()
    return nc


def _cm(v, nseg):
    """flat [-1] -> [128, nseg] with cm[p, s] = v[128s + p]"""
    return np.ascontiguousarray(np.asarray(v, np.float32).reshape(nseg, 128).T)


def prep_inputs(inputs):
    f32 = lambda x: np.ascontiguousarray(np.asarray(x, dtype=np.float32))
    bf = lambda x: np.ascontiguousarray(
        np.asarray(x, dtype=np.float32).astype(ml_dtypes.bfloat16))
    m = {}
    Wv, Wt = inputs["Wv"], inputs["Wtheta"]
    W1, W2 = inputs["W1"], inputs["W2"]
    for l in range(L):
        wv = np.asarray(Wv[l]).reshape(6, 128, E).transpose(1, 0, 2)
        wt = np.asarray(Wt[l]).reshape(6, 128, E).transpose(1, 0, 2)
        for c in range(2):
            m[f"wv{c}_{l}"] = bf(wv[:, 3 * c:3 * c + 3].reshape(128, 3 * E))
            m[f"wt{c}_{l}"] = bf(wt[:, 3 * c:3 * c + 3].reshape(128, 3 * E))
        w1 = np.asarray(W1[l], np.float32)              # [E, HID]
        s1 = np.abs(w1).max(axis=0) / F8LIM             # [HID]
        w1q = (w1 / s1).reshape(6, 128, HID).transpose(1, 0, 2)
        for c in range(6):
            m[f"w1{c}_{l}"] = np.ascontiguousarray(
                w1q[:, c].reshape(128, HID).astype(ml_dtypes.float8_e3m4))
        w2 = np.asarray(W2[l], np.float32)              # [HID, E]
        s2 = np.abs(w2).max(axis=0) / F8LIM             # [E]
        w2q = (w2 / s2).reshape(24, 128, E).transpose(1, 0, 2)
        for c in range(6):
            m[f"w2{c}_{l}"] = np.ascontiguousarray(
                w2q[:, 4 * c:4 * c + 4].reshape(128, 4 * E).astype(
                    ml_dtypes.float8_e3m4))
        pv = np.concatenate([
            _cm(inputs["ln1_s"][l], 6), _cm(inputs["ln1_b"][l], 6),
            _cm(inputs["ln2_s"][l], 6), _cm(inputs["ln2_b"][l], 6),
            _cm(s2, 6), _cm(inputs["b2"][l], 6),
            _cm(s1, 24), _cm(inputs["b1"][l], 24)], axis=1)
        m[f"pv{l}"] = f32(pv)
    wc1 = np.asarray(inputs["Wc1"]).reshape(6, 128, HID).transpose(1, 0, 2)
    for c in range(6):
        m[f"wc1{c}"] = bf(wc1[:, c].reshape(128, HID))
    wc2 = np.asarray(inputs["Wc2"]).reshape(24, 128, CLS).transpose(1, 0, 2)
    for c in range(8):
        m[f"wc2{c}"] = bf(wc2[:, 3 * c:3 * c + 3].reshape(128, 3 * CLS))
    m["fcm"] = f32(np.concatenate([
        _cm(inputs["lnf_s"], 6), _cm(inputs["lnf_b"], 6),
        _cm(inputs["bc1"], 24)], axis=1))
    m["fb"] = f32(np.asarray(inputs["bc2"]).reshape(1, CLS))
    m["identf"] = np.eye(128, dtype=np.float32)
    m["onesc"] = np.ones((128, 1), np.float32)
    m["onesr"] = np.ones((1, 128), np.float32)
    u0 = np.asarray(inputs["class_token"]).reshape(E) + \
        np.asarray(inputs["pos"]).reshape(-1, E)[-1]
    m["u0"] = _cm(u0, 6)
    return m


_CACHED = {}


def kernel(**inputs) -> np.ndarray:
    b = int(np.asarray(inputs["x"]).shape[0])
    in_map = prep_inputs(inputs)
    if "nc" not in _CACHED:
        _CACHED["nc"] = build_program()
    nc = _CACHED["nc"]
    r = run_bass_kernel_spmd(nc, [in_map], core_ids=[0])
    out = np.asarray(r.results[0]["out"]).reshape(1, CLS)
    return np.ascontiguousarray(np.broadcast_to(out, (b, CLS)).astype(np.float32))
